# revision 6
# baseline (speedup 1.0000x reference)
"""Trainium2 Bass kernel for NeuralGraphHidden (GNN message passing).

Full-input contract: kernel(**inputs) takes the complete unsharded arrays,
shards batch dim 0 across 8 NeuronCores (data parallel), runs one SPMD Bass
program, and reassembles the full output.

Math (per molecule, A=128 atoms, D=5 degree slots):
  deg[a]      = #(edges[a,:] != -1)
  neigh[a]    = atoms[a] + sum_d atoms[edges[a,d]]        (-1 -> no contribution)
  sbond[a]    = sum_d bonds[a,d]
  feat[a]     = [neigh[a], sbond[a], 1.0]                 (bias folded as feature)
  Z_d[a]      = feat[a] @ Waug[d]                          (Waug = [W;b])
  out[a]      = relu(Z_{deg[a]}[a])  if deg[a] < 5 else 0

On-chip formulation:
  ET'[a',a] = I + sum_d onehot(edges[a,d])[a']   built via DVE is_equal vs iota
  neighT    = atoms_chunk.T @ ET'                (TensorE, contracts atoms axis)
  deg+1     = ones.T-col-sums of ET'             (TensorE)
  bondsT    = 5 accumulating transposes          (TensorE, rhs=I)
  Z         = featT.T @ Waug                     (TensorE, 3 K-chunks x 5 degrees)
  select    = sum_d diag(deg==d) @ Z_d           (TensorE, PSUM-accumulated;
                                                  exact: relu and select commute)
  out       = relu(select)                       (ScalarE)
"""

import sys

sys.path.insert(0, "/opt/trn_rl_repo")

import numpy as np

B, A, D = 256, 128, 5
FA, FB, C = 256, 64, 256
F = FA + FB        # 320
FAUG = F + 1       # 321 (bias row)
NCORES = 8
BL = B // NCORES   # 32 molecules per core

_CACHE = {}


def _build_program():
    from contextlib import ExitStack

    import concourse.bass as bass
    import concourse.tile as tile
    from concourse import bacc, mybir

    f32 = mybir.dt.float32
    i32 = mybir.dt.int32
    AF = mybir.ActivationFunctionType
    OP = mybir.AluOpType

    nc = bacc.Bacc("TRN2", target_bir_lowering=False, debug=False,
                   num_devices=NCORES)

    atoms_d = nc.dram_tensor("atoms", [BL, A, FA], f32, kind="ExternalInput")
    bonds_d = nc.dram_tensor("bonds", [BL, A, D * FB], f32, kind="ExternalInput")
    # edge indices as f32, pre-replicated down the partition axis by the host
    edges_d = nc.dram_tensor("edges", [BL, A, A * D], f32, kind="ExternalInput")
    waug_d = nc.dram_tensor("waug", [D, FAUG, C], f32, kind="ExternalInput")
    ident_d = nc.dram_tensor("ident", [A, A], f32, kind="ExternalInput")
    iota_d = nc.dram_tensor("iota", [A, 1], f32, kind="ExternalInput")
    onesc_d = nc.dram_tensor("onesc", [A, 1], f32, kind="ExternalInput")
    onesr_d = nc.dram_tensor("onesr", [1, A], f32, kind="ExternalInput")
    out_d = nc.dram_tensor("out", [BL, A, C], f32, kind="ExternalOutput")

    atoms_ap = atoms_d.ap()
    bonds_ap = bonds_d.ap()
    edges_ap = edges_d.ap()
    out_ap = out_d.ap()

    with tile.TileContext(nc) as tc, ExitStack() as ctx:
        consts = ctx.enter_context(tc.tile_pool(name="consts", bufs=1))
        pin = ctx.enter_context(tc.tile_pool(name="pin", bufs=3))
        pbc = ctx.enter_context(tc.tile_pool(name="pbc", bufs=2))
        pet = ctx.enter_context(tc.tile_pool(name="pet", bufs=2))
        pfeat = ctx.enter_context(tc.tile_pool(name="pfeat", bufs=2))
        pmd = ctx.enter_context(tc.tile_pool(name="pmd", bufs=2))
        pz = ctx.enter_context(tc.tile_pool(name="pz", bufs=2))
        pout = ctx.enter_context(tc.tile_pool(name="pout", bufs=3))
        ps_f = ctx.enter_context(
            tc.tile_pool(name="ps_f", bufs=2, space="PSUM"))
        ps_c2 = ctx.enter_context(
            tc.tile_pool(name="ps_c2", bufs=2, space="PSUM"))
        ps_z = ctx.enter_context(
            tc.tile_pool(name="ps_z", bufs=1, space="PSUM"))
        ps_s = ctx.enter_context(
            tc.tile_pool(name="ps_s", bufs=1, space="PSUM"))

        # ---- one-time setup -------------------------------------------------
        ident = consts.tile([A, A], f32)
        nc.sync.dma_start(out=ident[:], in_=ident_d.ap()[:])
        iota_col = consts.tile([A, 1], f32)
        nc.sync.dma_start(out=iota_col[:], in_=iota_d.ap()[:])
        ones_col = consts.tile([A, 1], f32)
        nc.sync.dma_start(out=ones_col[:], in_=onesc_d.ap()[:])
        ones_row = consts.tile([1, A], f32)
        nc.sync.dma_start(out=ones_row[:], in_=onesr_d.ap()[:])

        # Weights resident in SBUF: chunk k holds rows [k*128, ...) of Waug
        # for all 5 degrees side by side: w_k[:, d*256:(d+1)*256].
        w0 = consts.tile([128, D * C], f32)
        w1 = consts.tile([128, D * C], f32)
        w2 = consts.tile([FAUG - 256, D * C], f32)  # 65 rows: 64 bond + bias
        for d in range(D):
            nc.sync.dma_start(out=w0[:, d * C:(d + 1) * C],
                              in_=waug_d.ap()[d, 0:128, :])
            nc.sync.dma_start(out=w1[:, d * C:(d + 1) * C],
                              in_=waug_d.ap()[d, 128:256, :])
            nc.sync.dma_start(out=w2[:, d * C:(d + 1) * C],
                              in_=waug_d.ap()[d, 256:FAUG, :])

        # ---- per molecule ---------------------------------------------------
        for bm in range(BL):
            atoms_sb = pin.tile([A, FA], f32)
            nc.sync.dma_start(out=atoms_sb[:], in_=atoms_ap[bm])
            bonds_sb = pin.tile([A, D * FB], f32)
            nc.sync.dma_start(out=bonds_sb[:], in_=bonds_ap[bm])

            # This molecule's 640 edge slots, replicated on every partition.
            bc_e = pbc.tile([A, A * D], f32)
            nc.sync.dma_start(out=bc_e[:], in_=edges_ap[bm])

            # cmp5[a', a*5+d] = (edges[a,d] == a')
            cmp5 = pbc.tile([A, A * D], f32)
            nc.vector.tensor_scalar(cmp5[:], bc_e[:], iota_col[:], None,
                                    OP.is_equal)
            # ET[a', a] = sum_d cmp5 ; ET' = ET + I (self term)
            et = pet.tile([A, A], f32)
            nc.vector.reduce_sum(
                et[:], cmp5.rearrange("p (a d) -> p a d", d=D),
                axis=mybir.AxisListType.X)
            etp = pet.tile([A, A], f32)
            nc.vector.tensor_add(etp[:], et[:], ident[:])

            # Neighbor+self sums (transposed) and deg+1, all in one PSUM tile:
            # cols [0:128] f-chunk0, [128:256] f-chunk1, [256:257] deg+1.
            pf = ps_f.tile([A, 257], f32)
            nc.tensor.matmul(pf[:, 0:128], atoms_sb[:, 0:128], etp[:])
            nc.tensor.matmul(pf[:, 128:256], atoms_sb[:, 128:256], etp[:])
            nc.tensor.matmul(pf[:, 256:257], etp[:], ones_col[:])

            featT01 = pfeat.tile([A, FA], f32)
            nc.scalar.copy(featT01[:], pf[:, 0:FA])
            degp1 = pfeat.tile([A, 1], f32)
            nc.scalar.copy(degp1[:], pf[:, 256:257])

            # Bond sums, transposed: 5 accumulating "transpose" matmuls.
            pc2 = ps_c2.tile([FB, A], f32)
            for d in range(D):
                nc.tensor.matmul(pc2[:], bonds_sb[:, d * FB:(d + 1) * FB],
                                 ident[:], start=(d == 0), stop=(d == D - 1))
            chunk2 = pfeat.tile([FAUG - 256, A], f32)
            nc.scalar.copy(chunk2[0:FB, :], pc2[:])
            nc.vector.tensor_copy(chunk2[FB:FB + 1, :], ones_row[:])

            # maskdiag_d = diag(deg == d), one fused DVE op per degree:
            # (I * (deg+1)) == (d+1): diagonal hits iff deg==d, off-diag 0!=d+1.
            md = pmd.tile([A, D * A], f32)
            for d in range(D):
                nc.vector.tensor_scalar(md[:, d * A:(d + 1) * A], ident[:],
                                        degp1[:], float(d + 1),
                                        OP.mult, OP.is_equal)

            # Dense: Z[:, d*256:(d+1)*256] = feat @ Waug[d], K split 128+128+65.
            pzt = ps_z.tile([A, D * C], f32)
            lhs = [featT01[:, 0:128], featT01[:, 128:256], chunk2[:]]
            rhs = [w0, w1, w2]
            groups = [(0, 512), (512, 1024), (1024, 1280)]
            for k in range(3):
                for g0, g1 in groups:
                    nc.tensor.matmul(pzt[:, g0:g1], lhs[k], rhs[k][:, g0:g1],
                                     start=(k == 0), stop=(k == 2))
            zsb = pz.tile([A, D * C], f32)
            for g0, g1 in groups:
                nc.scalar.copy(zsb[:, g0:g1], pzt[:, g0:g1])

            # Degree select: S = sum_d maskdiag_d @ Z_d (PSUM-accumulated).
            pst = ps_s.tile([A, C], f32)
            for d in range(D):
                nc.tensor.matmul(pst[:], md[:, d * A:(d + 1) * A],
                                 zsb[:, d * C:(d + 1) * C],
                                 start=(d == 0), stop=(d == D - 1))

            out_sb = pout.tile([A, C], f32)
            nc.scalar.activation(out_sb[:], pst[:], AF.Relu)
            nc.sync.dma_start(out=out_ap[bm], in_=out_sb[:])

    nc.compile()
    return nc


def _get_nc():
    if "nc" not in _CACHE:
        _CACHE["nc"] = _build_program()
    return _CACHE["nc"]


def _make_in_maps(atoms, bonds, edges, W, b):
    atoms = np.ascontiguousarray(np.asarray(atoms, dtype=np.float32))
    bonds = np.ascontiguousarray(np.asarray(bonds, dtype=np.float32))
    edges = np.asarray(edges)
    W = np.asarray(W, dtype=np.float32)
    b = np.asarray(b, dtype=np.float32)

    # f32 edge slots replicated down the partition axis (layout prep for the
    # on-chip one-hot compare; the DMA engines cannot zero-step partitions).
    edges_f = edges.reshape(B, A * D).astype(np.float32)
    edges_rep = np.ascontiguousarray(
        np.broadcast_to(edges_f[:, None, :], (B, A, A * D)))

    waug = np.ascontiguousarray(
        np.concatenate([W, b[:, None, :]], axis=1))           # (5, 321, 256)
    ident = np.eye(A, dtype=np.float32)
    iota = np.arange(A, dtype=np.float32).reshape(A, 1)
    onesc = np.ones((A, 1), dtype=np.float32)
    onesr = np.ones((1, A), dtype=np.float32)

    atoms8 = atoms.reshape(NCORES, BL, A, FA)
    bonds8 = bonds.reshape(NCORES, BL, A, D * FB)
    edges8 = edges_rep.reshape(NCORES, BL, A, A * D)

    return [
        {
            "atoms": atoms8[c],
            "bonds": bonds8[c],
            "edges": edges8[c],
            "waug": waug,
            "ident": ident,
            "iota": iota,
            "onesc": onesc,
            "onesr": onesr,
        }
        for c in range(NCORES)
    ]


def run_sharded(atoms, bonds, edges, W, b, trace=False):
    """Run on the 8 NeuronCores; returns (output, BassKernelResults)."""
    from concourse.bass_utils import run_bass_kernel_spmd

    nc = _get_nc()
    in_maps = _make_in_maps(atoms, bonds, edges, W, b)
    res = run_bass_kernel_spmd(nc, in_maps, list(range(NCORES)), trace=trace)
    out = np.concatenate([res.results[c]["out"] for c in range(NCORES)],
                         axis=0).reshape(B, A, C)
    return out, res


def kernel(atoms, bonds, edges, W, b):
    out, _ = run_sharded(atoms, bonds, edges, W, b)
    return out


# revision 9
# speedup vs baseline: 2.1885x; 2.1885x over previous
"""Trainium2 Bass kernel for NeuralGraphHidden (GNN message passing).

Full-input contract: kernel(**inputs) takes the complete unsharded arrays,
shards batch dim 0 across 8 NeuronCores (data parallel), runs one SPMD Bass
program, and reassembles the full output.

Math (per molecule, A=128 atoms, D=5 degree slots):
  deg[a]      = #(edges[a,:] != -1)
  neigh[a]    = atoms[a] + sum_d atoms[edges[a,d]]        (-1 -> no contribution)
  sbond[a]    = sum_d bonds[a,d]
  feat[a]     = [neigh[a], sbond[a], 1.0]                 (bias folded as feature)
  Z_d[a]      = feat[a] @ Waug[d]                          (Waug = [W;b])
  out[a]      = relu(Z_{deg[a]}[a])  if deg[a] < 5 else 0

On-chip formulation:
  ET'[a',a] = I + sum_d onehot(edges[a,d])[a']   built via DVE is_equal vs iota
  neighT    = atoms_chunk.T @ ET'                (TensorE, contracts atoms axis)
  deg+1     = ones.T-col-sums of ET'             (TensorE)
  bondsT    = 5 accumulating transposes          (TensorE, rhs=I)
  Z         = featT.T @ Waug                     (TensorE, 3 K-chunks x 5 degrees)
  select    = sum_d diag(deg==d) @ Z_d           (TensorE, PSUM-accumulated;
                                                  exact: relu and select commute)
  out       = relu(select)                       (ScalarE)
"""

import sys

sys.path.insert(0, "/opt/trn_rl_repo")

import numpy as np

B, A, D = 256, 128, 5
FA, FB, C = 256, 64, 256
F = FA + FB        # 320
FAUG = F + 1       # 321 (bias row)
NCORES = 8
BL = B // NCORES   # 32 molecules per core

_CACHE = {}


def _build_program():
    from contextlib import ExitStack

    import concourse.bass as bass
    import concourse.tile as tile
    from concourse import bacc, mybir

    f32 = mybir.dt.float32
    i32 = mybir.dt.int32
    AF = mybir.ActivationFunctionType
    OP = mybir.AluOpType
    # float32r operands: single-pass (TF32-like) PE matmul at 2x fp32 rate;
    # every matmul operand below is produced/declared as f32r to satisfy the
    # BIR verifier's rounding rule. One-hot/mask/count values are small
    # integers, exactly representable at reduced mantissa, so the gather and
    # degree-select logic stays exact.
    f32r = mybir.dt.float32r

    nc = bacc.Bacc("TRN2", target_bir_lowering=False, debug=False,
                   num_devices=NCORES)

    atoms_d = nc.dram_tensor("atoms", [BL, A, FA], f32r, kind="ExternalInput")
    bonds_d = nc.dram_tensor("bonds", [BL, A, D * FB], f32r,
                             kind="ExternalInput")
    # edge indices as f32, pre-replicated down the partition axis by the host
    edges_d = nc.dram_tensor("edges", [BL, A, A * D], f32, kind="ExternalInput")
    waug_d = nc.dram_tensor("waug", [D, FAUG, C], f32r, kind="ExternalInput")
    ident_d = nc.dram_tensor("ident", [A, A], f32, kind="ExternalInput")
    identr_d = nc.dram_tensor("identr", [A, A], f32r, kind="ExternalInput")
    iota_d = nc.dram_tensor("iota", [A, 1], f32, kind="ExternalInput")
    onesc_d = nc.dram_tensor("onesc", [A, 2], f32r, kind="ExternalInput")
    onesr_d = nc.dram_tensor("onesr", [1, A], f32, kind="ExternalInput")
    out_d = nc.dram_tensor("out", [BL, A, C], f32, kind="ExternalOutput")

    atoms_ap = atoms_d.ap()
    bonds_ap = bonds_d.ap()
    edges_ap = edges_d.ap()
    out_ap = out_d.ap()

    with tile.TileContext(nc) as tc, ExitStack() as ctx:
        consts = ctx.enter_context(tc.tile_pool(name="consts", bufs=1))
        pin = ctx.enter_context(tc.tile_pool(name="pin", bufs=3))
        pbc = ctx.enter_context(tc.tile_pool(name="pbc", bufs=2))
        pet = ctx.enter_context(tc.tile_pool(name="pet", bufs=2))
        pfeat = ctx.enter_context(tc.tile_pool(name="pfeat", bufs=2))
        pmd = ctx.enter_context(tc.tile_pool(name="pmd", bufs=2))
        pz = ctx.enter_context(tc.tile_pool(name="pz", bufs=2))
        pout = ctx.enter_context(tc.tile_pool(name="pout", bufs=3))
        ps_f = ctx.enter_context(
            tc.tile_pool(name="ps_f", bufs=2, space="PSUM"))
        ps_c2 = ctx.enter_context(
            tc.tile_pool(name="ps_c2", bufs=2, space="PSUM"))
        ps_z = ctx.enter_context(
            tc.tile_pool(name="ps_z", bufs=1, space="PSUM"))
        ps_s = ctx.enter_context(
            tc.tile_pool(name="ps_s", bufs=1, space="PSUM"))

        # ---- one-time setup -------------------------------------------------
        ident = consts.tile([A, A], f32)
        nc.sync.dma_start(out=ident[:], in_=ident_d.ap()[:])
        identr = consts.tile([A, A], f32r)
        nc.sync.dma_start(out=identr[:], in_=identr_d.ap()[:])
        iota_col = consts.tile([A, 1], f32)
        nc.sync.dma_start(out=iota_col[:], in_=iota_d.ap()[:])
        ones_col = consts.tile([A, 2], f32r)
        nc.sync.dma_start(out=ones_col[:], in_=onesc_d.ap()[:])
        ones_row = consts.tile([1, A], f32)
        nc.sync.dma_start(out=ones_row[:], in_=onesr_d.ap()[:])

        # Weights resident in SBUF: chunk k holds rows [k*128, ...) of Waug
        # for all 5 degrees side by side: w_k[:, d*256:(d+1)*256].
        w0 = consts.tile([128, D * C], f32r)
        w1 = consts.tile([128, D * C], f32r)
        w2 = consts.tile([FAUG - 256, D * C], f32r)  # 65 rows: 64 bond + bias
        for d in range(D):
            nc.sync.dma_start(out=w0[:, d * C:(d + 1) * C],
                              in_=waug_d.ap()[d, 0:128, :])
            nc.sync.dma_start(out=w1[:, d * C:(d + 1) * C],
                              in_=waug_d.ap()[d, 128:256, :])
            nc.sync.dma_start(out=w2[:, d * C:(d + 1) * C],
                              in_=waug_d.ap()[d, 256:FAUG, :])

        # ---- per molecule ---------------------------------------------------
        for bm in range(BL):
            atoms_sb = pin.tile([A, FA], f32r)
            nc.sync.dma_start(out=atoms_sb[:], in_=atoms_ap[bm])
            bonds_sb = pin.tile([A, D * FB], f32r)
            nc.sync.dma_start(out=bonds_sb[:], in_=bonds_ap[bm])

            # This molecule's 640 edge slots, replicated on every partition.
            bc_e = pbc.tile([A, A * D], f32)
            nc.sync.dma_start(out=bc_e[:], in_=edges_ap[bm])

            # cmp5[a', a*5+d] = (edges[a,d] == a')
            cmp5 = pbc.tile([A, A * D], f32)
            nc.vector.tensor_scalar(cmp5[:], bc_e[:], iota_col[:], None,
                                    OP.is_equal)
            # ET[a', a] = sum_d cmp5 ; ET' = ET + I (self term)
            et = pet.tile([A, A], f32)
            nc.vector.reduce_sum(
                et[:], cmp5.rearrange("p (a d) -> p a d", d=D),
                axis=mybir.AxisListType.X)
            etp = pet.tile([A, A], f32r)
            nc.vector.tensor_add(etp[:], et[:], ident[:])

            # Neighbor+self sums (transposed) and deg+1, all in one PSUM tile:
            # cols [0:128] f-chunk0, [128:256] f-chunk1, [256:257] deg+1
            # (257 is a duplicate: f32r matmul needs an even moving-free dim).
            pf = ps_f.tile([A, 258], f32)
            nc.tensor.matmul(pf[:, 0:128], atoms_sb[:, 0:128], etp[:])
            nc.tensor.matmul(pf[:, 128:256], atoms_sb[:, 128:256], etp[:])
            nc.tensor.matmul(pf[:, 256:258], etp[:], ones_col[:])

            featT01 = pfeat.tile([A, FA], f32r)
            nc.scalar.copy(featT01[:], pf[:, 0:FA])
            degp1 = pfeat.tile([A, 1], f32)
            nc.scalar.copy(degp1[:], pf[:, 256:257])

            # Bond sums, transposed: 5 accumulating "transpose" matmuls.
            pc2 = ps_c2.tile([FB, A], f32)
            for d in range(D):
                nc.tensor.matmul(pc2[:],
                                 bonds_sb[:, d * FB:(d + 1) * FB],
                                 identr[:], start=(d == 0),
                                 stop=(d == D - 1))
            chunk2 = pfeat.tile([FAUG - 256, A], f32r)
            nc.scalar.copy(chunk2[0:FB, :], pc2[:])
            nc.vector.tensor_copy(chunk2[FB:FB + 1, :], ones_row[:])

            # maskdiag_d = diag(deg == d), one fused DVE op per degree:
            # (I * (deg+1)) == (d+1): diagonal hits iff deg==d, off-diag 0!=d+1.
            md = pmd.tile([A, D * A], f32r)
            for d in range(D):
                nc.vector.tensor_scalar(md[:, d * A:(d + 1) * A], ident[:],
                                        degp1[:], float(d + 1),
                                        OP.mult, OP.is_equal)

            # Dense: Z[:, d*256:(d+1)*256] = feat @ Waug[d], K split 128+128+65.
            pzt = ps_z.tile([A, D * C], f32)
            lhs = [featT01[:, 0:128], featT01[:, 128:256], chunk2[:]]
            rhs = [w0, w1, w2]
            groups = [(0, 512), (512, 1024), (1024, 1280)]
            for k in range(3):
                for g0, g1 in groups:
                    nc.tensor.matmul(pzt[:, g0:g1], lhs[k],
                                     rhs[k][:, g0:g1],
                                     start=(k == 0), stop=(k == 2))
            zsb = pz.tile([A, D * C], f32r)
            for g0, g1 in groups:
                nc.scalar.copy(zsb[:, g0:g1], pzt[:, g0:g1])

            # Degree select: S = sum_d maskdiag_d @ Z_d (PSUM-accumulated).
            pst = ps_s.tile([A, C], f32)
            for d in range(D):
                nc.tensor.matmul(pst[:], md[:, d * A:(d + 1) * A],
                                 zsb[:, d * C:(d + 1) * C],
                                 start=(d == 0), stop=(d == D - 1))

            out_sb = pout.tile([A, C], f32)
            nc.scalar.activation(out_sb[:], pst[:], AF.Relu)
            nc.sync.dma_start(out=out_ap[bm], in_=out_sb[:])

    nc.compile()
    return nc


def _get_nc():
    if "nc" not in _CACHE:
        _CACHE["nc"] = _build_program()
    return _CACHE["nc"]


def _make_in_maps(atoms, bonds, edges, W, b):
    atoms = np.ascontiguousarray(np.asarray(atoms, dtype=np.float32))
    bonds = np.ascontiguousarray(np.asarray(bonds, dtype=np.float32))
    edges = np.asarray(edges)
    W = np.asarray(W, dtype=np.float32)
    b = np.asarray(b, dtype=np.float32)

    # f32 edge slots replicated down the partition axis (layout prep for the
    # on-chip one-hot compare; the DMA engines cannot zero-step partitions).
    edges_f = edges.reshape(B, A * D).astype(np.float32)
    edges_rep = np.ascontiguousarray(
        np.broadcast_to(edges_f[:, None, :], (B, A, A * D)))

    waug = np.ascontiguousarray(
        np.concatenate([W, b[:, None, :]], axis=1))           # (5, 321, 256)
    ident = np.eye(A, dtype=np.float32)
    iota = np.arange(A, dtype=np.float32).reshape(A, 1)
    onesc = np.ones((A, 1), dtype=np.float32)
    onesr = np.ones((1, A), dtype=np.float32)

    atoms8 = atoms.reshape(NCORES, BL, A, FA)
    bonds8 = bonds.reshape(NCORES, BL, A, D * FB)
    edges8 = edges_rep.reshape(NCORES, BL, A, A * D)

    return [
        {
            "atoms": atoms8[c],
            "bonds": bonds8[c],
            "edges": edges8[c],
            "waug": waug,
            "ident": ident,
            "identr": ident,
            "iota": iota,
            "onesc": np.ones((A, 2), dtype=np.float32),
            "onesr": onesr,
        }
        for c in range(NCORES)
    ]


def run_sharded(atoms, bonds, edges, W, b, trace=False):
    """Run on the 8 NeuronCores; returns (output, BassKernelResults)."""
    from concourse.bass_utils import run_bass_kernel_spmd

    nc = _get_nc()
    in_maps = _make_in_maps(atoms, bonds, edges, W, b)
    res = run_bass_kernel_spmd(nc, in_maps, list(range(NCORES)), trace=trace)
    out = np.concatenate([res.results[c]["out"] for c in range(NCORES)],
                         axis=0).reshape(B, A, C)
    return out, res


def kernel(atoms, bonds, edges, W, b):
    out, _ = run_sharded(atoms, bonds, edges, W, b)
    return out


# revision 12
# speedup vs baseline: 2.7039x; 1.2355x over previous
"""Trainium2 Bass kernel for NeuralGraphHidden (GNN message passing).

Full-input contract: kernel(**inputs) takes the complete unsharded arrays,
shards batch dim 0 across 8 NeuronCores (data parallel), runs one SPMD Bass
program, and reassembles the full output.

Math (per molecule, A=128 atoms, D=5 degree slots):
  deg[a]      = #(edges[a,:] != -1)
  neigh[a]    = atoms[a] + sum_d atoms[edges[a,d]]        (-1 -> no contribution)
  sbond[a]    = sum_d bonds[a,d]
  feat[a]     = [neigh[a], sbond[a], 1.0]                 (bias folded as feature)
  Z_d[a]      = feat[a] @ Waug[d]                          (Waug = [W;b])
  out[a]      = relu(Z_{deg[a]}[a])  if deg[a] < 5 else 0

On-chip formulation:
  ET'[a',a] = I + sum_d onehot(edges[a,d])[a']   built via DVE is_equal vs iota
  neighT    = atoms_chunk.T @ ET'                (TensorE, contracts atoms axis)
  deg+1     = ones.T-col-sums of ET'             (TensorE)
  bondsT    = 5 accumulating transposes          (TensorE, rhs=I)
  Z         = featT.T @ Waug                     (TensorE, 3 K-chunks x 5 degrees)
  select    = sum_d diag(deg==d) @ Z_d           (TensorE, PSUM-accumulated;
                                                  exact: relu and select commute)
  out       = relu(select)                       (ScalarE)
"""

import sys

sys.path.insert(0, "/opt/trn_rl_repo")

import numpy as np

B, A, D = 256, 128, 5
FA, FB, C = 256, 64, 256
F = FA + FB        # 320
FAUG = F + 1       # 321 (bias row)
NCORES = 8
BL = B // NCORES   # 32 molecules per core

_CACHE = {}


def _build_program():
    from contextlib import ExitStack

    import concourse.bass as bass
    import concourse.tile as tile
    from concourse import bacc, mybir

    f32 = mybir.dt.float32
    i32 = mybir.dt.int32
    AF = mybir.ActivationFunctionType
    OP = mybir.AluOpType
    # float32r operands: single-pass (TF32-like) PE matmul at 2x fp32 rate;
    # every matmul operand below is produced/declared as f32r to satisfy the
    # BIR verifier's rounding rule. One-hot/mask/count values are small
    # integers, exactly representable at reduced mantissa, so the gather and
    # degree-select logic stays exact.
    f32r = mybir.dt.float32r

    nc = bacc.Bacc("TRN2", target_bir_lowering=False, debug=False,
                   num_devices=NCORES)

    atoms_d = nc.dram_tensor("atoms", [BL, A, FA], f32r, kind="ExternalInput")
    bonds_d = nc.dram_tensor("bonds", [BL, A, D * FB], f32,
                             kind="ExternalInput")
    # edge indices as bf16 (exact for -1..127), host-replicated down partitions
    bf16 = mybir.dt.bfloat16
    edges_d = nc.dram_tensor("edges", [BL, A, A * D], bf16,
                             kind="ExternalInput")
    waug_d = nc.dram_tensor("waug", [D, FAUG, C], f32r, kind="ExternalInput")
    ident_d = nc.dram_tensor("ident", [A, A], f32, kind="ExternalInput")
    identr_d = nc.dram_tensor("identr", [A, A], f32r, kind="ExternalInput")
    iota_d = nc.dram_tensor("iota", [A, 1], f32, kind="ExternalInput")
    onesc_d = nc.dram_tensor("onesc", [A, 2], f32r, kind="ExternalInput")
    onesr_d = nc.dram_tensor("onesr", [1, A], f32, kind="ExternalInput")
    out_d = nc.dram_tensor("out", [BL, A, C], f32, kind="ExternalOutput")

    atoms_ap = atoms_d.ap()
    bonds_ap = bonds_d.ap()
    edges_ap = edges_d.ap()
    out_ap = out_d.ap()

    with tile.TileContext(nc) as tc, ExitStack() as ctx:
        consts = ctx.enter_context(tc.tile_pool(name="consts", bufs=1))
        pin = ctx.enter_context(tc.tile_pool(name="pin", bufs=3))
        pbc = ctx.enter_context(tc.tile_pool(name="pbc", bufs=2))
        pet = ctx.enter_context(tc.tile_pool(name="pet", bufs=2))
        pfeat = ctx.enter_context(tc.tile_pool(name="pfeat", bufs=2))
        pmd = ctx.enter_context(tc.tile_pool(name="pmd", bufs=2))
        pz = ctx.enter_context(tc.tile_pool(name="pz", bufs=2))
        pout = ctx.enter_context(tc.tile_pool(name="pout", bufs=3))
        ps_f = ctx.enter_context(
            tc.tile_pool(name="ps_f", bufs=2, space="PSUM"))
        ps_c2 = ctx.enter_context(
            tc.tile_pool(name="ps_c2", bufs=1, space="PSUM"))
        ps_z = ctx.enter_context(
            tc.tile_pool(name="ps_z", bufs=1, space="PSUM"))
        ps_s = ctx.enter_context(
            tc.tile_pool(name="ps_s", bufs=1, space="PSUM"))

        # ---- one-time setup -------------------------------------------------
        ident = consts.tile([A, A], f32)
        nc.sync.dma_start(out=ident[:], in_=ident_d.ap()[:])
        identr = consts.tile([A, A], f32r)
        nc.sync.dma_start(out=identr[:], in_=identr_d.ap()[:])
        iota_col = consts.tile([A, 1], f32)
        nc.sync.dma_start(out=iota_col[:], in_=iota_d.ap()[:])
        ones_col = consts.tile([A, 2], f32r)
        nc.sync.dma_start(out=ones_col[:], in_=onesc_d.ap()[:])
        ones_row = consts.tile([1, A], f32)
        nc.sync.dma_start(out=ones_row[:], in_=onesr_d.ap()[:])

        # Weights resident in SBUF: chunk k holds rows [k*128, ...) of Waug
        # for all 5 degrees side by side: w_k[:, d*256:(d+1)*256].
        w0 = consts.tile([128, D * C], f32r)
        w1 = consts.tile([128, D * C], f32r)
        w2 = consts.tile([FAUG - 256, D * C], f32r)  # 65 rows: 64 bond + bias
        for d in range(D):
            nc.sync.dma_start(out=w0[:, d * C:(d + 1) * C],
                              in_=waug_d.ap()[d, 0:128, :])
            nc.sync.dma_start(out=w1[:, d * C:(d + 1) * C],
                              in_=waug_d.ap()[d, 128:256, :])
            nc.sync.dma_start(out=w2[:, d * C:(d + 1) * C],
                              in_=waug_d.ap()[d, 256:FAUG, :])

        # ---- per molecule ---------------------------------------------------
        for bm in range(BL):
            atoms_sb = pin.tile([A, FA], f32r)
            nc.sync.dma_start(out=atoms_sb[:], in_=atoms_ap[bm])
            bonds_sb = pin.tile([A, D * FB], f32)
            nc.sync.dma_start(out=bonds_sb[:], in_=bonds_ap[bm])

            # This molecule's 640 edge slots, replicated on every partition.
            bc_e = pbc.tile([A, A * D], bf16)
            nc.sync.dma_start(out=bc_e[:], in_=edges_ap[bm])

            # cmp5[a', a*5+d] = (edges[a,d] == a')
            cmp5 = pbc.tile([A, A * D], bf16)
            nc.vector.tensor_scalar(cmp5[:], bc_e[:], iota_col[:], None,
                                    OP.is_equal)
            # ET[a', a] = sum_d cmp5 ; ET' = ET + I (self term)
            et = pet.tile([A, A], f32)
            nc.vector.reduce_sum(
                et[:], cmp5.rearrange("p (a d) -> p a d", d=D),
                axis=mybir.AxisListType.X)
            etp = pet.tile([A, A], f32r)
            nc.vector.tensor_add(etp[:], et[:], ident[:])

            # Neighbor+self sums (transposed) and deg+1, all in one PSUM tile:
            # cols [0:128] f-chunk0, [128:256] f-chunk1, [256:257] deg+1
            # (257 is a duplicate: f32r matmul needs an even moving-free dim).
            pf = ps_f.tile([A, 258], f32)
            nc.tensor.matmul(pf[:, 0:128], atoms_sb[:, 0:128], etp[:])
            nc.tensor.matmul(pf[:, 128:256], atoms_sb[:, 128:256], etp[:])
            nc.tensor.matmul(pf[:, 256:258], etp[:], ones_col[:])

            featT01 = pfeat.tile([A, FA], f32r)
            nc.scalar.copy(featT01[:], pf[:, 0:FA])
            degp1 = pfeat.tile([A, 1], f32)
            nc.vector.tensor_copy(degp1[:], pf[:, 256:257])

            # Bond sums on DVE (reduce over the degree axis), then one
            # transpose matmul to get (fb, a) layout.
            sumbond = pfeat.tile([A, FB], f32r)
            with nc.allow_low_precision(reason="f32r rounding of bond sums"):
                nc.vector.reduce_sum(
                    sumbond[:], bonds_sb.rearrange("p (d f) -> p f d", d=D),
                    axis=mybir.AxisListType.X)
            pc2 = ps_c2.tile([FB, A], f32)
            nc.tensor.matmul(pc2[:], sumbond[:], identr[:])
            chunk2 = pfeat.tile([FAUG - 256, A], f32r)
            nc.scalar.copy(chunk2[0:FB, :], pc2[:])
            nc.vector.tensor_copy(chunk2[FB:FB + 1, :], ones_row[:])

            # maskdiag_d = diag(deg == d), one fused DVE op per degree:
            # (I * (deg+1)) == (d+1): diagonal hits iff deg==d, off-diag 0!=d+1.
            md = pmd.tile([A, D * A], f32r)
            for d in range(D):
                nc.vector.tensor_scalar(md[:, d * A:(d + 1) * A], ident[:],
                                        degp1[:], float(d + 1),
                                        OP.mult, OP.is_equal)

            # Dense: Z[:, d*256:(d+1)*256] = feat @ Waug[d], K split 128+128+65.
            lhs = [featT01[:, 0:128], featT01[:, 128:256], chunk2[:]]
            rhs = [w0, w1, w2]
            groups = [(0, 512), (512, 1024), (1024, 1280)]
            zsb = pz.tile([A, D * C], f32r)
            for g0, g1 in groups:
                pzg = ps_z.tile([A, 512], f32, tag="pzg", bufs=4)
                nc.tensor.matmul(pzg[:, 0:g1 - g0], lhs[0], rhs[0][:, g0:g1],
                                 start=True, stop=False)
                nc.tensor.matmul(pzg[:, 0:g1 - g0], lhs[1], rhs[1][:, g0:g1],
                                 start=False, stop=False)
                nc.tensor.matmul(pzg[:, 0:g1 - g0], lhs[2], rhs[2][:, g0:g1],
                                 start=False, stop=True)
                nc.scalar.copy(zsb[:, g0:g1], pzg[:, 0:g1 - g0])

            # Degree select: S = sum_d maskdiag_d @ Z_d (PSUM-accumulated).
            pst = ps_s.tile([A, C], f32)
            for d in range(D):
                nc.tensor.matmul(pst[:], md[:, d * A:(d + 1) * A],
                                 zsb[:, d * C:(d + 1) * C],
                                 start=(d == 0), stop=(d == D - 1))

            out_sb = pout.tile([A, C], f32)
            nc.scalar.activation(out_sb[:], pst[:], AF.Relu)
            nc.sync.dma_start(out=out_ap[bm], in_=out_sb[:])

    nc.compile()
    return nc


def _get_nc():
    if "nc" not in _CACHE:
        _CACHE["nc"] = _build_program()
    return _CACHE["nc"]


def _make_in_maps(atoms, bonds, edges, W, b):
    atoms = np.ascontiguousarray(np.asarray(atoms, dtype=np.float32))
    bonds = np.ascontiguousarray(np.asarray(bonds, dtype=np.float32))
    edges = np.asarray(edges)
    W = np.asarray(W, dtype=np.float32)
    b = np.asarray(b, dtype=np.float32)

    # bf16 edge slots (exact for -1..127) replicated down the partition axis
    # (layout prep for the on-chip one-hot compare; DMA cannot zero-step
    # partitions).
    import ml_dtypes
    edges_f = edges.reshape(B, A * D).astype(ml_dtypes.bfloat16)
    edges_rep = np.ascontiguousarray(
        np.broadcast_to(edges_f[:, None, :], (B, A, A * D)))

    waug = np.ascontiguousarray(
        np.concatenate([W, b[:, None, :]], axis=1))           # (5, 321, 256)
    ident = np.eye(A, dtype=np.float32)
    iota = np.arange(A, dtype=np.float32).reshape(A, 1)
    onesc = np.ones((A, 1), dtype=np.float32)
    onesr = np.ones((1, A), dtype=np.float32)

    atoms8 = atoms.reshape(NCORES, BL, A, FA)
    bonds8 = bonds.reshape(NCORES, BL, A, D * FB)
    edges8 = edges_rep.reshape(NCORES, BL, A, A * D)

    return [
        {
            "atoms": atoms8[c],
            "bonds": bonds8[c],
            "edges": edges8[c],
            "waug": waug,
            "ident": ident,
            "identr": ident,
            "iota": iota,
            "onesc": np.ones((A, 2), dtype=np.float32),
            "onesr": onesr,
        }
        for c in range(NCORES)
    ]


def run_sharded(atoms, bonds, edges, W, b, trace=False):
    """Run on the 8 NeuronCores; returns (output, BassKernelResults)."""
    from concourse.bass_utils import run_bass_kernel_spmd

    nc = _get_nc()
    in_maps = _make_in_maps(atoms, bonds, edges, W, b)
    res = run_bass_kernel_spmd(nc, in_maps, list(range(NCORES)), trace=trace)
    out = np.concatenate([res.results[c]["out"] for c in range(NCORES)],
                         axis=0).reshape(B, A, C)
    return out, res


def kernel(atoms, bonds, edges, W, b):
    out, _ = run_sharded(atoms, bonds, edges, W, b)
    return out
